# revision 13
# baseline (speedup 1.0000x reference)
"""GroupQLinear Trainium2 kernel.

y = quantize_per_token_groupwise(x) @ W.T + bias

Sharding: pure data-parallel over tokens. x [4,2048,4096] -> 8192 tokens,
1024 tokens per core; weight/bias replicated (weight pre-transposed and
cast to bf16 on host); each core computes its y shard [1024, 4096]
(stored output-transposed [4096, 1024] for clean DMA, un-transposed on
host).

Quantization (per token, groups of 32 along H):
  delta   = clip(absmax_g, 1e-5)/127
  db      = max_g delta
  R_init  = clip(mean_g(db - delta)/4, 1e-8)
  e       = clip(floor((delta-db)/R_init), -7, 0)
  loss(r) = mean_g(delta - db - e*(r/63)*db)^2 is a quadratic in r;
            argmin over the 64-point grid == grid point nearest the
            parabola vertex rc* = sum((db-delta)*(-e)) / (db*sum(e^2)).
            (verified exact match vs explicit argmin on the real data)
  drec    = clip(db + e*(k/63)*db, 1e-5)
  q_x     = round(x/drec)*drec        (round = RNE, via +/- 1.5*2^23)
"""

import os
from contextlib import ExitStack

import numpy as np
import ml_dtypes

import concourse.bass as bass
import concourse.bacc as bacc
import concourse.tile as tile
from concourse import mybir
from concourse.bass_utils import run_bass_kernel_spmd

F32 = mybir.dt.float32
BF16 = mybir.dt.bfloat16
ALU = mybir.AluOpType
ACT = mybir.ActivationFunctionType

B, T, H, O = 4, 2048, 4096, 4096
NCORES = 8
TOK = B * T                 # 8192 tokens
TPC = TOK // NCORES         # 1024 tokens per core
GW = 32                     # group width
G = H // GW                 # 128 groups per token
QT = 128                    # tokens per quant tile
NQT = TPC // QT             # 8 quant tiles per core
MMT = 512                   # tokens per matmul moving group
NGRP = TPC // MMT           # 2 matmul groups per core
NKT = H // 128              # 32 k-tiles
NOT = O // 128              # 32 o-tiles
MAGIC = float(np.float32(1.5 * 2 ** 23))   # RNE rounding constant
INV127 = float(np.float32(1.0) / np.float32(127.0))
INV63 = float(np.float32(1.0) / np.float32(63.0))


def _bcast(a, b):
    """Broadcast AP a (with size-1 dims) against b's free dims."""
    a2, _ = bass.broadcast_tensor_aps(a, b)
    return a2


def build_kernel(ctx: ExitStack, tc: tile.TileContext, x_d, wt_d, bias_d,
                 ident_d, y_d):
    nc = tc.nc

    const_p = ctx.enter_context(tc.tile_pool(name="const", bufs=1))
    x_p = ctx.enter_context(tc.tile_pool(name="xin", bufs=3))
    v_p = ctx.enter_context(tc.tile_pool(name="vwork", bufs=2))
    qx_p = ctx.enter_context(tc.tile_pool(name="qx", bufs=2))
    qxt_p = ctx.enter_context(tc.tile_pool(name="qxt", bufs=1))
    sm_p = ctx.enter_context(tc.tile_pool(name="small", bufs=2))
    wt_p = ctx.enter_context(tc.tile_pool(name="wt", bufs=3))
    y_p = ctx.enter_context(tc.tile_pool(name="yout", bufs=3))
    ps_t = ctx.enter_context(tc.tile_pool(name="ps_tr", bufs=4, space="PSUM"))
    ps_m = ctx.enter_context(tc.tile_pool(name="ps_mm", bufs=4, space="PSUM"))

    ident = const_p.tile([128, 128], BF16, tag="ident")
    nc.sync.dma_start(ident[:], ident_d)
    bias_sb = const_p.tile([128, NOT], F32, tag="bias")
    nc.sync.dma_start(bias_sb[:], bias_d)
    magic_p = const_p.tile([128, 1], F32, tag="magic_p")
    nc.vector.memset(magic_p[:], MAGIC)
    magic_n = const_p.tile([128, 1], F32, tag="magic_n")
    nc.vector.memset(magic_n[:], -MAGIC)

    # one qxT buffer per matmul group: [h%128, h//128, t] bf16
    qxT = [qxt_p.tile([128, NKT, MMT], BF16, tag=f"qxT{g}", name=f"qxT{g}")
           for g in range(NGRP)]

    # ---------------- quantization ----------------
    for i in range(NQT):
        g = i // (MMT // QT)            # matmul group
        toff = (i % (MMT // QT)) * QT   # token offset within group

        xt = x_p.tile([128, H], F32, tag="xt")
        nc.sync.dma_start(xt[:], x_d[i * QT:(i + 1) * QT, :])
        xg = xt[:].rearrange("p (g w) -> p g w", w=GW)

        absm = sm_p.tile([128, G], F32, tag="absm")
        nc.vector.tensor_reduce(absm[:], xg, axis=mybir.AxisListType.X,
                                op=ALU.max, apply_absolute_value=True)
        delta = sm_p.tile([128, G], F32, tag="delta")
        nc.vector.tensor_scalar(delta[:], absm[:], 1e-5, INV127,
                                op0=ALU.max, op1=ALU.mult)
        db = sm_p.tile([128, 1], F32, tag="db")
        nc.vector.tensor_reduce(db[:], delta[:], axis=mybir.AxisListType.X,
                                op=ALU.max)
        db_g = _bcast(db[:], delta[:])

        # diff = db - delta (>=0), rsum = sum_g diff
        diff = sm_p.tile([128, G], F32, tag="diff")
        rsum = sm_p.tile([128, 1], F32, tag="rsum")
        nc.vector.scalar_tensor_tensor(diff[:], delta[:], -1.0, db_g,
                                       op0=ALU.mult, op1=ALU.add,
                                       accum_out=rsum[:])
        # R_init = max(rsum/512, 1e-8); rR = 1/R_init
        Rin = sm_p.tile([128, 1], F32, tag="Rin")
        nc.vector.tensor_scalar(Rin[:], rsum[:], 1.0 / 512.0, 1e-8,
                                op0=ALU.mult, op1=ALU.max)
        rR = sm_p.tile([128, 1], F32, tag="rR")
        nc.vector.reciprocal(rR[:], Rin[:])
        # u = (delta - db)*rR <= 0;  -floor(u) = RNE(diff*rR + 0.5)
        t05 = sm_p.tile([128, G], F32, tag="t05")
        nc.vector.tensor_scalar(t05[:], diff[:], rR[:], 0.5,
                                op0=ALU.mult, op1=ALU.add)
        rt = sm_p.tile([128, G], F32, tag="rt")
        nc.vector.tensor_scalar(rt[:], t05[:], MAGIC, MAGIC,
                                op0=ALU.add, op1=ALU.subtract)
        en = sm_p.tile([128, G], F32, tag="en")
        nc.vector.tensor_scalar(en[:], rt[:], 7.0, None, op0=ALU.min)
        # Ps = sum diff*en ; Qs = sum en*en
        tP = sm_p.tile([128, G], F32, tag="tP")
        Ps = sm_p.tile([128, 1], F32, tag="Ps")
        nc.vector.scalar_tensor_tensor(tP[:], en[:], 1.0, diff[:],
                                       op0=ALU.mult, op1=ALU.mult,
                                       accum_out=Ps[:])
        tQ = sm_p.tile([128, G], F32, tag="tQ")
        Qs = sm_p.tile([128, 1], F32, tag="Qs")
        nc.vector.scalar_tensor_tensor(tQ[:], en[:], 1.0, en[:],
                                       op0=ALU.mult, op1=ALU.mult,
                                       accum_out=Qs[:])
        # k = clip(rne(63 * Ps / max(db*Qs, 1e-30)), 0, 63)
        den = sm_p.tile([128, 1], F32, tag="den")
        nc.vector.tensor_scalar(den[:], Qs[:], db[:], 1e-30,
                                op0=ALU.mult, op1=ALU.max)
        rden = sm_p.tile([128, 1], F32, tag="rden")
        nc.vector.reciprocal(rden[:], den[:])
        kf = sm_p.tile([128, 1], F32, tag="kf")
        nc.vector.tensor_scalar(kf[:], Ps[:], rden[:], 63.0,
                                op0=ALU.mult, op1=ALU.mult)
        kr = sm_p.tile([128, 1], F32, tag="kr")
        nc.vector.tensor_scalar(kr[:], kf[:], MAGIC, MAGIC,
                                op0=ALU.add, op1=ALU.subtract)
        kk = sm_p.tile([128, 1], F32, tag="kk")
        nc.vector.tensor_scalar(kk[:], kr[:], 0.0, 63.0,
                                op0=ALU.max, op1=ALU.min)
        # bRn = -(k/63)*db ; drec = max(en*bRn + db, 1e-5); rs = 1/drec
        bRn = sm_p.tile([128, 1], F32, tag="bRn")
        nc.vector.tensor_scalar(bRn[:], kk[:], -INV63, db[:],
                                op0=ALU.mult, op1=ALU.mult)
        drec0 = sm_p.tile([128, G], F32, tag="drec0")
        nc.vector.scalar_tensor_tensor(drec0[:], en[:], bRn[:], db_g,
                                       op0=ALU.mult, op1=ALU.add)
        drec = sm_p.tile([128, G], F32, tag="drec")
        nc.vector.tensor_scalar(drec[:], drec0[:], 1e-5, None, op0=ALU.max)
        rs = sm_p.tile([128, G], F32, tag="rs")
        nc.vector.reciprocal(rs[:], drec[:])

        # v = x * rs (group-broadcast); round on Act; qx = v * drec -> bf16
        v = v_p.tile([128, H], F32, tag="v")
        vg = v[:].rearrange("p (g w) -> p g w", w=GW)
        rs3 = rs[:].rearrange("p (g o) -> p g o", o=1)
        nc.vector.tensor_tensor(vg, xg, _bcast(rs3, xg), op=ALU.mult)
        nc.scalar.activation(v[:], v[:], ACT.Identity, bias=magic_p[:])
        nc.scalar.activation(v[:], v[:], ACT.Identity, bias=magic_n[:])
        qx = qx_p.tile([128, H], BF16, tag="qx")
        qxg = qx[:].rearrange("p (g w) -> p g w", w=GW)
        drec3 = drec[:].rearrange("p (g o) -> p g o", o=1)
        nc.vector.tensor_tensor(qxg, vg, _bcast(drec3, vg), op=ALU.mult)

        # transpose 128x128 blocks into qxT[g][:, k, toff:toff+128]
        for k in range(NKT):
            pst = ps_t.tile([128, 128], BF16, tag="pst")
            nc.tensor.transpose(pst[:], qx[:, k * 128:(k + 1) * 128], ident[:])
            nc.scalar.copy(qxT[g][:, k, toff:toff + QT], pst[:])

    # ---------------- matmul ----------------
    for g in range(NGRP):
        for ot in range(NOT):
            wt = wt_p.tile([128, NKT, 128], BF16, tag="wt")
            nc.sync.dma_start(wt[:], wt_d[ot])
            ps = ps_m.tile([128, MMT], F32, tag="psmm")
            for k in range(NKT):
                nc.tensor.matmul(ps[:], wt[:, k, :], qxT[g][:, k, :],
                                 start=(k == 0), stop=(k == NKT - 1))
            yb = y_p.tile([128, MMT], F32, tag="yb")
            nc.scalar.activation(yb[:], ps[:], ACT.Identity,
                                 bias=bias_sb[:, ot:ot + 1], scale=1.0)
            nc.sync.dma_start(
                y_d[ot * 128:(ot + 1) * 128, g * MMT:(g + 1) * MMT], yb[:])


_NC_CACHE = {}


def _build_nc():
    if "nc" in _NC_CACHE:
        return _NC_CACHE["nc"]
    nc = bacc.Bacc("TRN2", target_bir_lowering=False, debug=False)
    x_d = nc.dram_tensor("x", [TPC, H], F32, kind="ExternalInput").ap()
    wt_d = nc.dram_tensor("wt", [NOT, 128, NKT, 128], BF16,
                          kind="ExternalInput").ap()
    bias_d = nc.dram_tensor("bias", [128, NOT], F32, kind="ExternalInput").ap()
    ident_d = nc.dram_tensor("ident", [128, 128], BF16,
                             kind="ExternalInput").ap()
    y_d = nc.dram_tensor("yt", [O, TPC], F32, kind="ExternalOutput").ap()
    with tile.TileContext(nc) as tc, ExitStack() as ctx:
        build_kernel(ctx, tc, x_d, wt_d, bias_d, ident_d, y_d)
    nc.compile()
    _NC_CACHE["nc"] = nc
    return nc


def prep_inputs(x: np.ndarray, weight: np.ndarray, bias: np.ndarray):
    """Host-side shard/layout prep -> list of 8 in_maps."""
    xs = np.ascontiguousarray(x.reshape(TOK, H), dtype=np.float32)
    # wt[ot, p, k, m] = W[128*ot + m, 128*k + p]
    wt = weight.reshape(NOT, 128, NKT, 128)          # [ot, m, k, p]
    wt = np.ascontiguousarray(wt.transpose(0, 3, 2, 1)).astype(
        ml_dtypes.bfloat16)
    bias_h = np.ascontiguousarray(
        bias.reshape(NOT, 128).T, dtype=np.float32)   # [p, ot]
    ident = np.eye(128, dtype=ml_dtypes.bfloat16)
    in_maps = []
    for c in range(NCORES):
        in_maps.append({
            "x": xs[c * TPC:(c + 1) * TPC],
            "wt": wt,
            "bias": bias_h,
            "ident": ident,
        })
    return in_maps


def run(x, weight, bias, trace=False, **kw):
    nc = _build_nc()
    in_maps = prep_inputs(np.asarray(x), np.asarray(weight), np.asarray(bias))
    res = run_bass_kernel_spmd(nc, in_maps, core_ids=list(range(NCORES)),
                               trace=trace, **kw)
    outs = [res.results[c]["yt"] for c in range(NCORES)]
    y = np.concatenate([o.T for o in outs], axis=0)   # [TOK, O]
    return y.reshape(B, T, O).astype(np.float32), res


def kernel(x: np.ndarray, weight: np.ndarray, bias: np.ndarray) -> np.ndarray:
    y, _ = run(x, weight, bias, trace=False)
    return y
